# revision 16
# baseline (speedup 1.0000x reference)
"""Trainium2 Bass kernel for nn_BatchHighOrderActivation.

Reference semantics (per batch b, channel g):
    sort the ARITY=4 values x = X[b,g,:], build barycentric coefficients from
    the sorted gaps, gather params rows by reverse-cumsum bitmasks, contract.

Sort/gather-free reformulation (multilinear simplex / Lovasz form):
    out[b,g,:] = sum_{m=1..15} w[b,g,m] * params[g,m,:]
    w[m] = relu( min_{i in m} x_i - max_{i not in m} x_i )   for m != 15
    w[15] = min_i x_i                                        (no relu)

Kernel structure per core (pure batch data-parallel sharding, 512 rows/core),
fp16 internal compute and fp16 DRAM output (host casts back to fp32; total
error ~7e-4 absmax-relative, well under the 2e-3 gate):
  - host: X de-interleaved to fp16 arity-planes grouped in b-tile PAIRS
          ([pair, p, a, t, g]); params expanded to an fp16 block-diagonal
          table (8 channels/group, K-order (m,gl))
  - DVE : subset min/max tree, subtractions and m15 quad-min computed once
          per PAIR of b-tiles (FD=1024 halves the per-op overhead share) into
          persistent pair-W tiles [p, t, (q,m,gl)]; relu per (t, q-half).
          The first pair is processed in g-halves for an earlier pipeline
          start, and the second pair's tree ops are EMISSION-INTERLEAVED
          between the first pair's W^T-evac copies so the DVE never idles
          waiting on PE transposes.
  - PE  : transpose W 128x128 chunks via fp16 identity matmul (16 chunks per
          PSUM tile), then block-diag fp16 matmul (K=(m,gl)=128), fp32 PSUM
  - DVE : W^T evacuation (fp16 2x) for 12 of 16 tiles; ACT takes b-tile 0's
          (DVE is tree-busy early, ACT idle)
  - ACT : out evacuation PSUM->SBUF with fused fp32->fp16 cast; DVE takes
          the last couple (tail)
  - DMA : input loads ride SWDGE (gpsimd) queues; fp16 stores on HWDGE (sync)
NOTE: GpSimd tensor ops contend for SBUF ports and drop concurrent DVE
throughput ~3x (measured); GpSimd does only pre-flight memsets + DMA here.
History: 122.5us baseline -> 118.3us (v7: fp16 out, paired tree, g-halved
head) -> this version.
"""

import numpy as np
from contextlib import ExitStack

import concourse.bass as bass
import concourse.mybir as mybir
import concourse.tile as tile
from concourse import bacc
from concourse.bass_utils import run_bass_kernel_spmd
from concourse.masks import make_identity

F32 = mybir.dt.float32
F16 = mybir.dt.float16
NCORES = 8
B, G, A, O = 4096, 512, 4, 32
BS = B // NCORES        # 512 batch rows per core
NQ = G // 8             # 64 channel groups of 8

_PAIRS = [(0, 1), (0, 2), (0, 3), (1, 2), (1, 3), (2, 3)]
_TRIPLES = [(0, 1, 2), (0, 1, 3), (0, 2, 3), (1, 2, 3)]

_cached_nc = None


def _build_program():
    nc = bacc.Bacc("TRN2", target_bir_lowering=False, debug=False, num_devices=NCORES)

    # x: [pair*128 + p, (a, t, g)] fp16 — b-tile pairs interleaved on host
    x_d = nc.dram_tensor("x", [BS // 2, A * 2 * G], F16, kind="ExternalInput").ap()
    pbd_d = nc.dram_tensor("pbd", [128, NQ * 256], F16, kind="ExternalInput").ap()
    out_d = nc.dram_tensor("out", [BS, G * O], F16, kind="ExternalOutput").ap()

    with ExitStack() as ctx:
        tc = ctx.enter_context(tile.TileContext(nc))
        persist = ctx.enter_context(tc.tile_pool(name="persist", bufs=1))
        treep = ctx.enter_context(tc.tile_pool(name="tree", bufs=1))
        lhsp = ctx.enter_context(tc.tile_pool(name="lt", bufs=6))
        stgp = ctx.enter_context(tc.tile_pool(name="stg", bufs=3))
        ptp = ctx.enter_context(tc.tile_pool(name="pt", bufs=1, space="PSUM"))
        pmp = ctx.enter_context(tc.tile_pool(name="pm", bufs=3, space="PSUM"))

        pbd = persist.tile([128, NQ * 256], F16)
        identity = persist.tile([128, 128], F16)
        pls = [persist.tile([128, A, 2, G], F16, name=f"pl{p}") for p in range(2)]
        # persistent pair-W tiles: [p, t, (q, m, gl)]
        wps = [persist.tile([128, 2, NQ * 128], F16, name=f"wp{p}") for p in range(2)]

        # first half of pair0 first, so the tree can start ASAP
        GH = G // 2
        x0_4d = x_d[0:128, :].rearrange("p (a t g) -> p a t g", a=A, t=2)
        nc.gpsimd.dma_start(pls[0][:, :, :, 0:GH], x0_4d[:, :, :, 0:GH])
        nc.gpsimd.dma_start(pls[0][:, :, :, GH:G], x0_4d[:, :, :, GH:G])
        nc.gpsimd.dma_start(
            pls[1][:].rearrange("p a t g -> p (a t g)"), x_d[128:256, :]
        )
        nc.gpsimd.dma_start(pbd[:], pbd_d[:])
        make_identity(nc, identity[:])
        # one-time m0-column zeroing of both persistent W tiles (the m0 matmul
        # contribution is killed by the zero params rows, but must be finite)
        for wp in wps:
            wv = wp.rearrange("p t (q m gl) -> p t q m gl", m=16, gl=8)
            nc.gpsimd.memset(wv[:, :, :, 0, :], 0.0)

        # ---- emission helpers --------------------------------------------
        def tree_ops(pr, ga, gb):
            """Yield closures emitting the pair-tree + subs + relu for a
            g-range of pair pr (both b-tiles at once, FD=2*(gb-ga))."""
            pl, wp = pls[pr], wps[pr]
            qa, qb = ga // 8, gb // 8
            s2 = [pl[:, i, :, ga:gb] for i in range(A)]
            tr = treep.tile([128, 20, 2, G], F16, tag="tree",
                            name=f"tr_{pr}_{ga}")
            slot = [0]
            mn, mx = {}, {}

            def alloc():
                ap = tr[:, slot[0], :, ga:gb]
                slot[0] += 1
                return ap

            def tt(dst, a, b, op):
                return lambda: nc.vector.tensor_tensor(dst, a, b, op)

            for (i, j) in _PAIRS:
                mn[(i, j)] = alloc()
                yield tt(mn[(i, j)], s2[i], s2[j], mybir.AluOpType.min)
            for (i, j) in _PAIRS:
                mx[(i, j)] = alloc()
                yield tt(mx[(i, j)], s2[i], s2[j], mybir.AluOpType.max)
            for (i, j, k) in _TRIPLES:
                mn[(i, j, k)] = alloc()
                yield tt(mn[(i, j, k)], mn[(i, j)], s2[k], mybir.AluOpType.min)
            for (i, j, k) in _TRIPLES:
                mx[(i, j, k)] = alloc()
                yield tt(mx[(i, j, k)], mx[(i, j)], s2[k], mybir.AluOpType.max)

            def sub_ap(S):
                return s2[S[0]] if len(S) == 1 else mn[S]

            def sup_ap(Cm):
                return s2[Cm[0]] if len(Cm) == 1 else mx[Cm]

            wv4 = wp.rearrange("p t (q m gl) -> p t q m gl", m=16, gl=8)
            wvr = wp.rearrange("p t (q r) -> p t q r", r=128)
            for m in range(1, 15):
                S = tuple(i for i in range(A) if (m >> i) & 1)
                Cm = tuple(i for i in range(A) if not ((m >> i) & 1))
                yield tt(wv4[:, :, qa:qb, m, :], sub_ap(S), sup_ap(Cm),
                         mybir.AluOpType.subtract)
            yield tt(wv4[:, :, qa:qb, 15, :], mn[(0, 1, 2)], s2[3],
                     mybir.AluOpType.min)
            for t in range(2):
                for q0 in range(qa, qb, 32):
                    dst = wvr[:, t, q0:q0 + 32, 8:120]
                    yield (lambda d=dst:
                           nc.vector.tensor_scalar_max(d, d, 0.0))

        def qg_unit(pr, t, qg):
            """Emit one 16-q group: transposes, W^T evac, matmuls, out evac,
            store."""
            bt = pr * 2 + t
            wt = wps[pr][:, t, :]
            pt = ptp.tile([128, 16 * 128], F16, tag="pt")
            for j in range(16):
                q = qg * 16 + j
                nc.tensor.transpose(
                    pt[:, j * 128:(j + 1) * 128],
                    wt[:, q * 128:(q + 1) * 128],
                    identity[:],
                )
            lt = lhsp.tile([128, 16 * 128], F16, tag="lt")
            if bt == 0:
                nc.scalar.copy(lt[:], pt[:])       # ACT: DVE tree-busy early
            else:
                nc.vector.tensor_copy(lt[:], pt[:])

            stg = stgp.tile([128, 16 * 256], F16, tag="stg")
            for seg in range(4):
                pm = pmp.tile([128, 1024], F32, tag="pm")
                for j2 in range(4):
                    j = seg * 4 + j2
                    q = qg * 16 + j
                    nc.tensor.matmul(
                        pm[:, j2 * 256:(j2 + 1) * 256],
                        lt[:, j * 128:(j + 1) * 128],
                        pbd[:, q * 256:(q + 1) * 256],
                        start=True,
                        stop=True,
                    )
                dst = stg[:, seg * 1024:(seg + 1) * 1024]
                if bt == 3 and qg == 3 and seg >= 2:
                    nc.vector.tensor_copy(dst, pm[:])   # tail help
                else:
                    nc.scalar.copy(dst, pm[:])
            q0 = qg * 16
            nc.sync.dma_start(
                out_d[bt * 128:(bt + 1) * 128, q0 * 256:(q0 + 16) * 256],
                stg[:],
            )

        def drain(gen, n=None):
            """Emit up to n ops from gen (all if n is None). True if empty."""
            k = 0
            for op in gen:
                op()
                k += 1
                if n is not None and k >= n:
                    return False
            return True

        # ---- schedule ----------------------------------------------------
        # pair0 half 0 (fast pipeline start)
        drain(tree_ops(0, 0, GH))
        # pair0 half 1 tree interleaved with half-0 qg units
        g_h1 = tree_ops(0, GH, G)
        units_h0 = [(0, 0), (1, 0), (0, 1), (1, 1)]        # (t, qg)
        for (t, qg) in units_h0:
            drain(g_h1, 10)
            qg_unit(0, t, qg)
        drain(g_h1)
        # pair1 tree interleaved with pair0's half-1 qg units
        g_p1 = tree_ops(1, 0, G)
        units_h1 = [(0, 2), (1, 2), (0, 3), (1, 3)]
        for (t, qg) in units_h1:
            drain(g_p1, 11)
            qg_unit(0, t, qg)
        drain(g_p1)
        # pair1 qg units (no more tree work to interleave)
        for (t, qg) in [(0, 0), (1, 0), (0, 1), (1, 1),
                        (0, 2), (1, 2), (0, 3), (1, 3)]:
            qg_unit(1, t, qg)

    nc.compile()
    return nc


def _get_program():
    global _cached_nc
    if _cached_nc is None:
        _cached_nc = _build_program()
    return _cached_nc


def _make_inputs(X, params):
    X = np.ascontiguousarray(X, dtype=np.float32)
    params = np.ascontiguousarray(params, dtype=np.float32)
    P4 = params.reshape(NQ, 8, 16, O)                 # [q, gl, m, o]
    # block-diag table: pbd[m*8+gl, q*256 + gl*32 + o] = params[8q+gl, m, o]
    Pb = np.zeros((16, 8, NQ, 8, O), np.float32)
    for gl in range(8):
        Pb[1:, gl, :, gl, :] = P4[:, gl, 1:, :].transpose(1, 0, 2)
    pbd = np.ascontiguousarray(Pb.reshape(128, NQ * 256).astype(np.float16))
    # X per core: [512b, G, A] -> [pair, p, a, t, g] fp16 planes
    Xp = (X.reshape(NCORES, 2, 2, 128, G, A)          # [c, pair, t, p, g, a]
            .transpose(0, 1, 3, 5, 2, 4)              # [c, pair, p, a, t, g]
            .astype(np.float16)
            .reshape(NCORES, 256, A * 2 * G))
    Xp = np.ascontiguousarray(Xp)
    in_maps = [
        {"x": Xp[c], "pbd": pbd}
        for c in range(NCORES)
    ]
    return in_maps


def kernel(X, params):
    nc = _get_program()
    in_maps = _make_inputs(X, params)
    res = run_bass_kernel_spmd(nc, in_maps, list(range(NCORES))).results
    out = np.concatenate(
        [res[c]["out"].astype(np.float32).reshape(BS, G, O) for c in range(NCORES)],
        axis=0,
    )
    return out


def kernel_traced(X, params):
    """Like kernel() but also returns the BassKernelResults (profile info)."""
    nc = _get_program()
    in_maps = _make_inputs(X, params)
    br = run_bass_kernel_spmd(nc, in_maps, list(range(NCORES)), trace=True)
    out = np.concatenate(
        [br.results[c]["out"].astype(np.float32).reshape(BS, G, O)
         for c in range(NCORES)],
        axis=0,
    )
    return out, br
